# revision 25
# baseline (speedup 1.0000x reference)
"""Trainium2 Bass kernel for nn_Attention_aggregator (B=8, N=4096, F=128, E=128).

Sharding: data-parallel over batch - one batch element per NeuronCore (8 cores).
Each core computes, for its batch b:
    att  = x @ x.T                        [N, N]
    att  = where(adj==0, -9999999, att)
    sm   = softmax(att, axis=-1)
    comb = sm @ x                         [N, F]
    out  = relu(concat([x, comb], -1) @ W.T)      [N, E]

Device decomposition (transposed orientation; contraction of the aggregation
matmul lands on partitions; attention symmetry makes transposed logits free):
    E^T[m, r] = exp(att[m, r] - 80)
    diagonal of att killed in PSUM by an accumulating (-30000*I) @ I matmul
    P^T = E^T * adjT  (adjT int8 in HBM, DMA-cast to int16 in SBUF)
    [S2 | S1] = P^T.T @ [x | 1]   (ones column => row-sum in column F)
    comb = (ev*S2 + coef*x) / (ev*S1 + coef)  with d = ||x_r||^2,
        ev = exp(-adj_rr*max(0, d-110)),  coef = adj_rr*exp(min(d-80, 30))
    out = relu([x, comb] @ W.T)

Perf structure vs the naive version:
  - adjacency stored int8 in HBM (16MB/core instead of 64MB), SWDGE DMA casts
    int8->int16 on the way into SBUF
  - logits accumulate into [128, 3, 512] PSUM groups (3 j-blocks = 3 banks,
    double buffered = 6 banks) so exp runs as one ACTIVATE over FD=1536
    (amortizes the ~352-cycle ACT instruction overhead)
  - mask applied as one [128,1536] bf16*int16 tensor_tensor (DVE 2x mode)
  - psumC packed into 2 banks: C0 holds quads t=0..2 ([128,387]), C1 holds
    quad t=3 plus scratch subviews for the epilogue transposes/out-matmuls
"""

import sys

for _p in ("/opt/trn_rl_repo", "/root/.axon_site/_ro/trn_rl_repo"):
    if _p not in sys.path:
        sys.path.append(_p)

import numpy as np

import concourse.bass as bass
import concourse.mybir as mybir
from concourse import bacc
from concourse.tile import TileContext
from concourse.masks import make_identity
from concourse.bass_utils import run_bass_kernel_spmd

F32 = mybir.dt.float32
BF16 = mybir.dt.bfloat16
I16 = mybir.dt.int16
I8 = mybir.dt.int8

B, N, F, E = 8, 4096, 128, 128
RC = 512               # r-chunk width (one PSUM bank of fp32)
NB = N // 128          # 32 m-blocks
NRC = N // RC          # 8 r-chunks
T = RC // 128          # 4 sub-blocks per r-chunk
EXP_BIAS = -80.0

# j-block group sizes per rc sweep (2 PSUM banks per group, triple buffered)
GROUPS = [3] * 10 + [2]
assert sum(GROUPS) == NB

_CACHED = {}


def _build():
    nc = bacc.Bacc("TRN2", target_bir_lowering=False, debug=False, num_devices=B)
    x_d = nc.dram_tensor("x", [128, NB, F], F32, kind="ExternalInput").ap()
    adjt_d = nc.dram_tensor("adjt", [N, N], I8, kind="ExternalInput").ap()
    adjd_d = nc.dram_tensor("adjd", [128, NB], F32, kind="ExternalInput").ap()
    dsq_d = nc.dram_tensor("dsq", [128, NB], F32, kind="ExternalInput").ap()
    w_d = nc.dram_tensor("w", [E, 2 * F], F32, kind="ExternalInput").ap()
    out_d = nc.dram_tensor("out", [N, E], F32, kind="ExternalOutput").ap()

    x_v = x_d  # host-shuffled to [128, NB, F] (m = o*128 + p)
    adjt_v = adjt_d.rearrange("(o p) c -> p o c", p=128)    # [128, NB, N] int8
    w_v = w_d.rearrange("e (h f) -> e h f", h=2)            # [128, 2, F]
    out_v = out_d.rearrange("(o p) e -> p o e", p=128)      # [128, NB, E]

    with TileContext(nc) as tc:
        with (
            tc.tile_pool(name="singles", bufs=1) as singles,
            tc.tile_pool(name="adjrc", bufs=6) as adjrc_pool,
            tc.tile_pool(name="xtmp", bufs=2) as xtmp_pool,
            tc.tile_pool(name="et", bufs=3) as e_pool,
            tc.tile_pool(name="pt", bufs=6) as p_pool,
            tc.tile_pool(name="small", bufs=12) as small,
            tc.tile_pool(name="sc", bufs=2) as sc_pool,
            tc.tile_pool(name="outp", bufs=6) as out_pool,
            tc.tile_pool(name="psumA", bufs=2, space="PSUM") as psum_a,
            tc.tile_pool(name="psumC", bufs=1, space="PSUM") as psum_c,
        ):
            # ---------------- setup ----------------
            # small input DMAs first (parallel with x load)
            w_sb = singles.tile([128, 2, F], F32)
            nc.scalar.dma_start(out=w_sb[:], in_=w_v)
            adjd_sb = singles.tile([128, NB], F32)
            nc.scalar.dma_start(out=adjd_sb[:], in_=adjd_d)
            d_sb = singles.tile([128, NB], F32)
            nc.scalar.dma_start(out=d_sb[:], in_=dsq_d)

            expb = singles.tile([128, 1], F32)
            nc.vector.memset(expb[:], EXP_BIAS)

            ident = singles.tile([128, 128], F32)
            make_identity(nc, ident)
            ident_bf = singles.tile([128, 128], BF16)
            nc.vector.tensor_copy(ident_bf[:], ident[:])
            negbig_bf = singles.tile([128, 128], BF16)
            nc.vector.tensor_scalar_mul(negbig_bf[:], ident_bf[:], -30000.0)

            # PSUM scratch for setup/epilogue transposes and out-matmuls lives
            # inside the psumC banks (subviews; quads only use cols 0:387/0:129)
            sc0ps = psum_c.tile([128, RC], F32, name="C0", tag="C0")
            sc1ps = psum_c.tile([128, RC], F32, name="C1", tag="C1")

            def psT_view(k):
                # [128,128] bf16 transpose target, alternating bank C0/C1
                if k % 2 == 0:
                    return sc0ps[:, 400:464].bitcast(BF16)
                return sc1ps[:, 256:320].bitcast(BF16)

            # x load pipelined per chunk with xb copy + xt transposes
            # (x is only kept in bf16: quads rhs, logits stationary, epilogue)
            xb_sb = singles.tile([128, NB, F + 4], BF16)
            xt_sb = singles.tile([128, NB, 128], BF16)
            nchunk = 2
            cw = NB // nchunk
            xtmps = []
            for c in range(nchunk):
                x_tmp = xtmp_pool.tile([128, cw, F], F32, name="x_tmp")
                nc.sync.dma_start(out=x_tmp[:],
                                  in_=x_v[:, c * cw:(c + 1) * cw, :])
                xtmps.append(x_tmp)
            for c in range(nchunk):
                x_tmp = xtmps[c]
                nc.vector.tensor_copy(xb_sb[:, c * cw:(c + 1) * cw, :F],
                                      x_tmp[:])
                nc.vector.memset(xb_sb[:, c * cw:(c + 1) * cw, F:F + 1], 1.0)
                for j in range(c * cw, (c + 1) * cw):
                    psb = psT_view(j)
                    nc.tensor.transpose(psb, xb_sb[:, j, 0:128], ident_bf[:])
                    nc.vector.tensor_copy(xt_sb[:, j, :], psb)

            # W^T fp32 halves [f part, e free]
            wb_sb = singles.tile([128, 2, F], BF16)
            nc.vector.tensor_copy(wb_sb[:], w_sb[:])
            wt_sb = singles.tile([128, 2, E], BF16)
            for h in range(2):
                psb = psT_view(h)
                nc.tensor.transpose(psb, wb_sb[:, h, :], ident_bf[:])
                nc.vector.tensor_copy(wt_sb[:, h, :], psb)

            # ev = exp(-adj_rr*max(0, d-110)); coef = adj_rr*exp(min(d-80, 30))
            ev_sb = singles.tile([128, NB], F32)
            coef_sb = singles.tile([128, NB], F32)
            t1 = small.tile([128, NB], F32, tag="cor")
            nc.vector.tensor_scalar(t1[:], d_sb[:], -110.0, 0.0,
                                    mybir.AluOpType.add, mybir.AluOpType.max)
            t2 = small.tile([128, NB], F32, tag="cor")
            nc.vector.tensor_tensor(t2[:], t1[:], adjd_sb[:], mybir.AluOpType.mult)
            nc.scalar.activation(ev_sb[:], t2[:],
                                 mybir.ActivationFunctionType.Exp, scale=-1.0)
            t3 = small.tile([128, NB], F32, tag="cor")
            nc.vector.tensor_scalar(t3[:], d_sb[:], -80.0, 30.0,
                                    mybir.AluOpType.add, mybir.AluOpType.min)
            t4 = small.tile([128, NB], F32, tag="cor")
            nc.scalar.activation(t4[:], t3[:], mybir.ActivationFunctionType.Exp)
            nc.vector.tensor_tensor(coef_sb[:], t4[:], adjd_sb[:],
                                    mybir.AluOpType.mult)

            # ---------------- main loop ----------------
            LAG = 4
            pending = []   # (rc, g, j0, gsz, pt_tile, sc0ps, sc1ps)

            def emit_quads(item):
                rc_, g_, j0_, gsz_, pt_, c0_, c1_ = item
                for jj in range(gsz_):
                    j = j0_ + jj
                    for t in range(T):
                        if t < 3:
                            outap = c0_[:, t * 129:t * 129 + 129]
                        else:
                            outap = c1_[:, 0:129]
                        nc.tensor.matmul(
                            outap,
                            pt_[:, jj, t * 128:(t + 1) * 128],
                            xb_sb[:, j, 0:F + 1],
                            start=(j == 0 and t in (0, 3)),
                            stop=(j == NB - 1 and t in (2, 3)),
                            skip_group_check=True)

            def emit_epilogue(rc_, c0_, c1_, trange=range(T), sc=None):
                # PSUM scratch subviews must come from THIS rc's C0/C1 tiles
                # (correct pool generation for dependency tracking)
                def psT_v(k):
                    if k % 2 == 0:
                        return c0_[:, 400:464].bitcast(BF16)
                    return c1_[:, 256:320].bitcast(BF16)

                if sc is None:
                    sc0 = sc_pool.tile([128, 387], F32, tag="sc0")
                    sc1 = sc_pool.tile([128, 129], F32, tag="sc1")
                    # psumC evacuation on the Scalar engine: it is idle at rc
                    # boundaries, and this frees C0/C1 for the next rc sooner
                    nc.scalar.copy(sc0[:], c0_[:, 0:387])
                    nc.scalar.copy(sc1[:], c1_[:, 0:129])
                else:
                    sc0, sc1 = sc
                for t in trange:
                    blk = rc_ * T + t
                    if t < 3:
                        S2 = sc0[:, t * 129:t * 129 + 128]
                        S1 = sc0[:, t * 129 + 128:t * 129 + 129]
                    else:
                        S2 = sc1[:, 0:128]
                        S1 = sc1[:, 128:129]
                    evb = ev_sb[:, blk:blk + 1]
                    cfb = coef_sb[:, blk:blk + 1]
                    den = small.tile([128, 1], F32, tag="den")
                    nc.vector.scalar_tensor_tensor(
                        den[:], S1, evb, cfb,
                        mybir.AluOpType.mult, mybir.AluOpType.add)
                    rden = small.tile([128, 1], F32, tag="rden")
                    nc.vector.reciprocal(rden[:], den[:])
                    xs = small.tile([128, F], BF16, tag="xs")
                    nc.vector.tensor_scalar_mul(xs[:], xb_sb[:, blk, 0:F], cfb)
                    cu = small.tile([128, F], F32, tag="cu")
                    nc.vector.scalar_tensor_tensor(
                        cu[:], S2, evb, xs[:],
                        mybir.AluOpType.mult, mybir.AluOpType.add)
                    cn = small.tile([128, F], BF16, tag="cn")
                    nc.vector.tensor_scalar_mul(cn[:], cu[:], rden[:])

                    psT = psT_v(t)
                    nc.tensor.transpose(psT, cn[:], ident_bf[:])
                    cnT = small.tile([128, F], BF16, tag="cnT")
                    nc.vector.tensor_copy(cnT[:], psT)

                    psF = c1_[:, 384:512]
                    nc.tensor.matmul(psF, xt_sb[:, blk, :], wt_sb[:, 0, :],
                                     start=True, stop=False,
                                     skip_group_check=True)
                    nc.tensor.matmul(psF, cnT[:], wt_sb[:, 1, :],
                                     start=False, stop=True,
                                     skip_group_check=True)
                    ot = out_pool.tile([128, E], F32)
                    nc.vector.tensor_relu(ot[:], psF)
                    nc.sync.dma_start(out=out_v[:, blk, :], in_=ot[:])
                return (sc0, sc1)

            # flat group list: (rc, g, j0, gsz)
            glist = []
            for rc in range(NRC):
                j0 = 0
                for g, gsz in enumerate(GROUPS):
                    glist.append((rc, g, j0, gsz))
                    j0 += gsz
            NG = len(glist)

            # adjacency: one cast-DMA per group (int8 HBM -> int16 SBUF),
            # prefetched PF groups ahead
            PF = 3
            adj_tiles = {}

            def issue_adj(i_):
                if i_ >= NG:
                    return
                rc_, _, j0_, gsz_ = glist[i_]
                adjg = adjrc_pool.tile([128, 3, RC], I16, name="adjg")
                nc.gpsimd.dma_start(
                    out=adjg[:, 0:gsz_, :],
                    in_=adjt_v[:, j0_:j0_ + gsz_, rc_ * RC:(rc_ + 1) * RC])
                adj_tiles[i_] = adjg

            for _i in range(PF):
                issue_adj(_i)

            c0_cur, c1_cur = sc0ps, sc1ps
            epi_tail = []   # deferred epilogue half: (rc, c0, c1, sc)
            for i, (rc, g, j0, gsz) in enumerate(glist):
                # 0. flush the deferred epilogue half (emitted before any new
                # quads touch the next-generation C0/C1 tiles, keeping all
                # same-bank ops in program order)
                while epi_tail:
                    rc_, c0_, c1_, sc_ = epi_tail.pop(0)
                    emit_epilogue(rc_, c0_, c1_, trange=(2, 3), sc=sc_)

                # 1. lagged quads first: keeps PE fed while exp/mask of newer
                # groups are still in flight (avoids head-of-line stalls).
                # Shrink the lag near the end so the drain tail is short.
                lag_now = LAG if i < NG - LAG - 2 else 1
                while len(pending) > lag_now:
                    item = pending.pop(0)
                    emit_quads(item)
                    if item[1] == len(GROUPS) - 1:
                        sc_ = emit_epilogue(item[0], item[5], item[6],
                                            trange=(0, 1))
                        epi_tail.append((item[0], item[5], item[6], sc_))

                # 2. prefetch adjacency for group i+PF
                issue_adj(i + PF)

                # 3. logits for group i
                if g == 0 and rc > 0:
                    c0_cur = psum_c.tile([128, RC], F32, name="C0", tag="C0")
                    c1_cur = psum_c.tile([128, RC], F32, name="C1", tag="C1")
                psA = psum_a.tile([128, 3, RC], F32, name="psA", tag="grp")
                for jj in range(gsz):
                    j = j0 + jj
                    diag = rc * T <= j < (rc + 1) * T
                    nc.tensor.matmul(psA[:, jj, :], xt_sb[:, j, :],
                                     xt_sb[:, rc * T:(rc + 1) * T, :],
                                     start=True, stop=not diag,
                                     skip_group_check=True)
                    if diag:
                        off = (j - rc * T) * 128
                        nc.tensor.matmul(psA[:, jj, off:off + 128],
                                         negbig_bf[:], ident_bf[:],
                                         start=False, stop=True,
                                         skip_group_check=True)

                # 4. exp + mask
                et = e_pool.tile([128, 3, RC], BF16, name="et")
                nc.scalar.activation(et[:, 0:gsz, :], psA[:, 0:gsz, :],
                                     mybir.ActivationFunctionType.Exp,
                                     bias=expb[:])
                pt = p_pool.tile([128, 3, RC], BF16, name="pt")
                adjg = adj_tiles.pop(i)
                nc.vector.tensor_tensor(
                    pt[:, 0:gsz, :], et[:, 0:gsz, :], adjg[:, 0:gsz, :],
                    mybir.AluOpType.mult)

                pending.append((rc, g, j0, gsz, pt, c0_cur, c1_cur))

            while epi_tail:
                rc_, c0_, c1_, sc_ = epi_tail.pop(0)
                emit_epilogue(rc_, c0_, c1_, trange=(2, 3), sc=sc_)
            while pending:
                item = pending.pop(0)
                emit_quads(item)
                if item[1] == len(GROUPS) - 1:
                    emit_epilogue(item[0], item[5], item[6])

    nc.compile()
    return nc


def _get_nc():
    if "nc" not in _CACHED:
        _CACHED["nc"] = _build()
    return _CACHED["nc"]


def kernel(**inputs) -> np.ndarray:
    x_all = np.asarray(inputs["node_features"], dtype=np.float32)   # [B, N, F]
    adj_all = np.asarray(inputs["adj_list"])                        # [B, N, N] int32
    W = np.asarray(inputs["W"], dtype=np.float32)                   # [E, 2F]

    nc = _get_nc()
    in_maps = []
    for b in range(B):
        adjt = np.ascontiguousarray(adj_all[b].T.astype(np.int8))
        diag = np.ascontiguousarray(np.diagonal(adj_all[b])).astype(np.float32)
        adjd = np.ascontiguousarray(diag.reshape(NB, 128).T)
        dsq = (x_all[b] * x_all[b]).sum(-1).astype(np.float32)
        dsq = np.ascontiguousarray(dsq.reshape(NB, 128).T)
        xshuf = np.ascontiguousarray(
            x_all[b].reshape(NB, 128, F).transpose(1, 0, 2))
        in_maps.append({
            "x": xshuf,
            "adjt": adjt,
            "adjd": adjd,
            "dsq": dsq,
            "w": W,
        })

    res = run_bass_kernel_spmd(nc, in_maps, core_ids=list(range(B)))
    out = np.stack([res.results[b]["out"] for b in range(B)], axis=0)
    return out.astype(np.float32, copy=False)
